# revision 1
# baseline (speedup 1.0000x reference)
"""Bass/Tile Trainium2 kernel for DiffMultiHeadedAttention.

Sharding: data-parallel over batch (B=2), tensor-parallel over heads
(16 heads -> 4 heads per core), 8 cores total. Each core computes the
QKV projections for its 4 heads, two-branch causal attention with the
differential combine + per-head RMS norm, and its partial output
projection; the host sums the 4 per-core partials of each batch.

Device-side layout: everything runs in the "transposed" orientation
(head-dim / key-pos on SBUF partitions) so every matmul consumes
operands in their natural layout. Softmax runs without max-subtraction
(scores are O(5) here, far from fp32 exp overflow); denominators are
computed with ones-vector matmuls on the PE, and per-column broadcasts
(1/denom, rms factor) are materialized with K=1 outer-product matmuls.
"""

import math
import sys

import numpy as np

sys.path.insert(0, "/opt/trn_rl_repo")

import ml_dtypes  # noqa: E402

import concourse.bass as bass  # noqa: E402
import concourse.tile as tile  # noqa: E402
from concourse import mybir  # noqa: E402
from concourse.bass_utils import run_bass_kernel_spmd  # noqa: E402

B, T, D = 2, 2048, 2048
H = 16
HD = 128
HALF = 64
DEPTH = 12
LAMBDA_INIT = 0.8 - 0.6 * math.exp(-0.3 * DEPTH)
EPS = 1e-5
SCALE = 1.0 / math.sqrt(HALF)

HPC = 4          # heads per core
M = HPC * HD     # 512: per-core projection width
N_CORES = 8

BF16 = mybir.dt.bfloat16
F32 = mybir.dt.float32
AF = mybir.ActivationFunctionType


def _split_sync_waits(nc: bass.Bass, limit: int = 1) -> int:
    """The walrus build in this container rejects instructions carrying
    more than one sem wait (setupSyncWait: "Too many sync wait
    commands"). Move excess waits onto same-engine nops inserted just
    before the instruction — the engine stalls at the nops instead, so
    semantics are unchanged."""
    import bass_rust as _br

    ctr = 0
    for fn in nc.m.functions:
        for blk in fn.blocks:
            insts = blk.instructions
            out = []
            changed = False
            for ins in insts:
                si = ins.sync_info
                waits = list(si.on_wait) if si is not None else []
                if len(waits) > limit and str(ins.engine) != "EngineType.Unassigned":
                    changed = True
                    for w in waits[:-limit]:
                        ctr += 1
                        nop = _br.InstNoOp(name=f"waitsplit-{ctr}", ins=[], outs=[])
                        nop.engine = ins.engine
                        nop.sync_info = _br.SyncInfo(on_wait=[w], on_update=[])
                        nc.register_instruction(nop, overwrite=True)
                        out.append(nop)
                    ins.sync_info = _br.SyncInfo(
                        on_wait=waits[-limit:], on_update=list(si.on_update)
                    )
                out.append(ins)
            if changed:
                insts[:] = out
    return ctr


def build_bass(lam: float, t: int = T) -> bass.Bass:
    """Emit the per-core program. `t` is the sequence length (2048 in
    production; smaller for simulator checks). Requires t % 512 == 0."""
    assert t % 512 == 0
    njb = t // 512     # tq blocks of 512
    nkt = t // 128     # tk tiles of 128
    nkd = D // 128     # contraction tiles for the projections (16)

    nc = bass.Bass()

    xtq = nc.dram_tensor("xtq", [D, t], BF16, kind="ExternalInput")
    xtk = nc.dram_tensor("xtk", [D, t], BF16, kind="ExternalInput")
    xtv = nc.dram_tensor("xtv", [D, t], BF16, kind="ExternalInput")
    wq = nc.dram_tensor("wq", [D, M], BF16, kind="ExternalInput")
    wk = nc.dram_tensor("wk", [D, M], BF16, kind="ExternalInput")
    wv = nc.dram_tensor("wv", [D, M], BF16, kind="ExternalInput")
    wo = nc.dram_tensor("wo", [M, D], BF16, kind="ExternalInput")
    masks = nc.dram_tensor("masks", [128, 128], BF16, kind="ExternalInput")
    rmsv = nc.dram_tensor("rmsv", [1, 128], F32, kind="ExternalInput")
    out = nc.dram_tensor("out", [t, D], F32, kind="ExternalOutput")

    with tile.TileContext(nc) as tc:
        with (
            tc.tile_pool(name="persist", bufs=1) as persist,
            tc.tile_pool(name="consts", bufs=1) as consts,
        ):
            ones_b = consts.tile([128, 1], BF16)
            nc.gpsimd.memset(ones_b[:], 1.0)
            onesr_f = consts.tile([1, 128], F32)
            nc.gpsimd.memset(onesr_f[:], 1.0)
            lam_f = consts.tile([1, 128], F32)
            nc.gpsimd.memset(lam_f[:], float(lam))
            rms_t = consts.tile([1, 128], F32)
            nc.sync.dma_start(rms_t[:], rmsv[:])
            eps_t = consts.tile([1, 1], F32)
            nc.gpsimd.memset(eps_t[:], EPS)
            mask_t = consts.tile([128, 128], BF16)
            nc.sync.dma_start(mask_t[:], masks[:])

            # persistent activations
            qt = persist.tile([128, HPC, njb, 512], BF16)   # [hd, h, jb, tq]
            kt = persist.tile([128, HPC, njb, 512], BF16)   # [hd, h, jb, tk]
            v = persist.tile([128, nkt, M], BF16)           # [t, ktile, m]
            attn = persist.tile([128, HPC, njb, 512], BF16)  # [hd, h, jb, tq]

            # ---- Phase A: projections ----
            with (
                tc.tile_pool(name="wp", bufs=2) as wpool,
                tc.tile_pool(name="xp", bufs=3) as xpool,
                tc.tile_pool(name="pa", bufs=4, space=bass.MemorySpace.PSUM) as pa,
            ):
                for wdram, xdram, dest, mode in (
                    (wv, xtv, v, "N"),
                    (wk, xtk, kt, "T"),
                    (wq, xtq, qt, "T"),
                ):
                  with nc.named_scope(f"proj_{mode}_{wdram.name}"):
                      w_sb = wpool.tile([128, nkd, M], BF16, tag="w")
                      nc.sync.dma_start(
                          w_sb[:], wdram.rearrange("(k p) m -> p k m", p=128)
                      )
                      for jb in range(njb):
                          x_sb = xpool.tile([128, nkd, 512], BF16, tag="xt")
                          nc.sync.dma_start(
                              x_sb[:],
                              xdram[:, 512 * jb : 512 * (jb + 1)].rearrange(
                                  "(k p) t -> p k t", p=128
                              ),
                          )
                          if mode == "T":
                              for mt in range(HPC):
                                  ps = pa.tile([128, 512], F32, tag="pa")
                                  for k in range(nkd):
                                      nc.tensor.matmul(
                                          ps[:],
                                          lhsT=w_sb[:, k, 128 * mt : 128 * (mt + 1)],
                                          rhs=x_sb[:, k, :],
                                          start=(k == 0),
                                          stop=(k == nkd - 1),
                                      )
                                  nc.vector.tensor_copy(dest[:, mt, jb, :], ps[:])
                          else:
                              for tw in range(4):
                                  ps = pa.tile([128, 512], F32, tag="pa")
                                  for k in range(nkd):
                                      nc.tensor.matmul(
                                          ps[:],
                                          lhsT=x_sb[:, k, 128 * tw : 128 * (tw + 1)],
                                          rhs=w_sb[:, k, :],
                                          start=(k == 0),
                                          stop=(k == nkd - 1),
                                      )
                                  nc.vector.tensor_copy(v[:, 4 * jb + tw, :], ps[:])

            # ---- Phase B+C: attention + interleaved output projection ----
            # The per-iteration epilogue (reciprocals, broadcasts, rms) is
            # emitted one iteration LATE so the next iteration's matmuls sit
            # ahead of it in the in-order PE queue — the PE no longer stalls
            # on the DVE reciprocal chain at every (h, j) boundary.
            with (
                tc.tile_pool(name="ep", bufs=2) as epool,
                tc.tile_pool(name="ft", bufs=2) as ftmp,
                tc.tile_pool(name="sm", bufs=2) as smtmp,
                tc.tile_pool(name="wop", bufs=1) as wop,
                tc.tile_pool(name="ob", bufs=2) as ob,
                tc.tile_pool(name="pss", bufs=2, space=bass.MemorySpace.PSUM) as pss,
                tc.tile_pool(name="psa", bufs=3, space=bass.MemorySpace.PSUM) as psa,
                tc.tile_pool(name="psd", bufs=1, space=bass.MemorySpace.PSUM) as psd,
            ):
                wo_sb = wop.tile([128, HPC, D], BF16)
                nc.sync.dma_start(wo_sb[:], wo.rearrange("(h p) n -> p h n", p=128))

                def emit_epi(h, j, att1, att2, dps):
                    sc_epi = nc.enter_named_scope("att_epi", False)
                    m1 = ftmp.tile([128, 512], F32, tag="m1")
                    nc.vector.tensor_copy(m1[:], att1[:])
                    m2 = ftmp.tile([128, 512], F32, tag="m2")
                    nc.vector.tensor_copy(m2[:], att2[:])
                    recip1 = smtmp.tile([1, 512], F32, tag="recip1")
                    nc.vector.reciprocal(recip1[:], dps[0:1, :])
                    recip2 = smtmp.tile([1, 512], F32, tag="recip2")
                    nc.vector.reciprocal(recip2[:], dps[32:33, :])
                    rb1 = psa.tile([128, 512], F32, tag="att")
                    nc.tensor.matmul(
                        rb1[:], lhsT=onesr_f[:], rhs=recip1[:],
                        start=True, stop=True,
                    )
                    nc.vector.tensor_mul(m1[:], m1[:], rb1[:])
                    rb2 = psa.tile([128, 512], F32, tag="att")
                    nc.tensor.matmul(
                        rb2[:], lhsT=lam_f[:], rhs=recip2[:],
                        start=True, stop=True,
                    )
                    nc.vector.tensor_mul(m2[:], m2[:], rb2[:])
                    comb = ftmp.tile([128, 512], F32, tag="comb")
                    nc.vector.tensor_sub(comb[:], m1[:], m2[:])
                    sq = ftmp.tile([128, 512], BF16, tag="sq")
                    nc.vector.tensor_mul(sq[:], comb[:], comb[:])
                    ssq = psa.tile([1, 512], F32, tag="att")
                    nc.tensor.matmul(
                        ssq[:], lhsT=ones_b[:], rhs=sq[:], start=True, stop=True
                    )
                    std = smtmp.tile([1, 512], F32, tag="std")
                    nc.scalar.activation(
                        std[:], ssq[:], AF.Sqrt, scale=1.0 / 128.0, bias=eps_t[:]
                    )
                    fac = smtmp.tile([1, 512], F32, tag="fac")
                    nc.vector.reciprocal(fac[:], std[:])
                    fb = psa.tile([128, 512], F32, tag="att")
                    nc.tensor.matmul(
                        fb[:], lhsT=rms_t[:], rhs=fac[:], start=True, stop=True
                    )
                    nc.vector.tensor_mul(attn[:, h, j, :], comb[:], fb[:])
                    nc.leave_named_scope("att_epi", sc_epi[0], False)

                def emit_outproj(j):
                    sc_o = nc.enter_named_scope("outproj", False)
                    for tw in range(4):
                        tt = 4 * j + tw
                        o_sb = ob.tile([128, D], F32, tag="o")
                        for db in range(4):
                            po = psa.tile([128, 512], F32, tag="att")
                            for h in range(HPC):
                                nc.tensor.matmul(
                                    po[:],
                                    lhsT=attn[:, h, j, 128 * tw : 128 * (tw + 1)],
                                    rhs=wo_sb[:, h, 512 * db : 512 * (db + 1)],
                                    start=(h == 0),
                                    stop=(h == HPC - 1),
                                )
                            nc.vector.tensor_copy(
                                o_sb[:, 512 * db : 512 * (db + 1)], po[:]
                            )
                        nc.sync.dma_start(out[128 * tt : 128 * (tt + 1), :], o_sb[:])
                    nc.leave_named_scope("outproj", sc_o[0], False)

                for j in range(njb):
                    for h in range(HPC):
                        ntk = 4 * j + 4
                        # [hd, tk-tile, branch, tq]
                        ea = epool.tile([128, nkt, 2, 512], BF16, tag="e")
                        att1 = psa.tile([128, 512], F32, tag="att")
                        att2 = psa.tile([128, 512], F32, tag="att")
                        dps = psd.tile([64, 512], F32, tag="d")
                        for i2 in range(ntk):
                            r = i2 - 4 * j
                            off = 128 * r if r > 0 else 0
                            # both branches share one 2-bank psum group so
                            # exp batches into one ACTIVATE over [128,1024]
                            sps = pss.tile([128, 2, 512], F32, tag="s")
                            sc_s = nc.enter_named_scope("att_s", False)
                            nc.tensor.matmul(
                                sps[:, 0, :],
                                lhsT=kt[0:64, h, i2 // 4, 128 * (i2 % 4) : 128 * (i2 % 4 + 1)],
                                rhs=qt[0:64, h, j, :],
                                start=True,
                                stop=True,
                            )
                            nc.tensor.matmul(
                                sps[:, 1, :],
                                lhsT=kt[64:128, h, i2 // 4, 128 * (i2 % 4) : 128 * (i2 % 4 + 1)],
                                rhs=qt[64:128, h, j, :],
                                start=True,
                                stop=True,
                            )
                            nc.leave_named_scope("att_s", sc_s[0], False)
                            sc_e = nc.enter_named_scope("att_exp", False)
                            nc.scalar.activation(
                                ea[:, i2, :, :], sps[:], AF.Exp, scale=SCALE
                            )
                            nc.leave_named_scope("att_exp", sc_e[0], False)
                            sc_m = nc.enter_named_scope("att_mask", False)
                            if r >= 0:
                                nc.vector.tensor_mul(
                                    ea[:, i2, 0, off : off + 128],
                                    ea[:, i2, 0, off : off + 128],
                                    mask_t[:],
                                )
                                nc.vector.tensor_mul(
                                    ea[:, i2, 1, off : off + 128],
                                    ea[:, i2, 1, off : off + 128],
                                    mask_t[:],
                                )
                            nc.leave_named_scope("att_mask", sc_m[0], False)
                            sc_a = nc.enter_named_scope("att_av", False)
                            nc.tensor.matmul(
                                att1[:, off:512],
                                lhsT=v[:, i2, 128 * h : 128 * (h + 1)],
                                rhs=ea[:, i2, 0, off:512],
                                start=(i2 == 0),
                                stop=(i2 == ntk - 1),
                            )
                            nc.tensor.matmul(
                                att2[:, off:512],
                                lhsT=v[:, i2, 128 * h : 128 * (h + 1)],
                                rhs=ea[:, i2, 1, off:512],
                                start=(i2 == 0),
                                stop=(i2 == ntk - 1),
                            )
                            nc.leave_named_scope("att_av", sc_a[0], False)
                            sc_d = nc.enter_named_scope("att_d", False)
                            nc.tensor.matmul(
                                dps[0:1, off:512],
                                lhsT=ones_b[:],
                                rhs=ea[:, i2, 0, off:512],
                                start=(i2 == 0),
                                stop=(i2 == ntk - 1),
                                tile_position=(0, 0),
                            )
                            nc.tensor.matmul(
                                dps[32:33, off:512],
                                lhsT=ones_b[:],
                                rhs=ea[:, i2, 1, off:512],
                                start=(i2 == 0),
                                stop=(i2 == ntk - 1),
                                tile_position=(0, 32),
                            )
                            nc.leave_named_scope("att_d", sc_d[0], False)

                        emit_epi(h, j, att1, att2, dps)
                        if h == HPC - 1:
                            emit_outproj(j)

    nsplit = _split_sync_waits(nc)
    return nc


def make_masks() -> np.ndarray:
    tk = np.arange(128)[:, None]
    tq = np.arange(128)[None, :]
    return (tq >= tk).astype(ml_dtypes.bfloat16)


def prep_inputs(query, key_in, value, Wq, Wk, Wv, Wo, rms_weight, t=T):
    """Build the 8 per-core input maps (host-side sharding + layout prep)."""
    bf = ml_dtypes.bfloat16
    masks_np = make_masks()
    rmsv_np = (
        np.asarray(rms_weight, np.float32) * (1.0 - LAMBDA_INIT)
    ).reshape(1, 128)
    xt = {}
    for b in range(B):
        xt[("q", b)] = np.ascontiguousarray(np.asarray(query[b]).T).astype(bf)
        xt[("k", b)] = np.ascontiguousarray(np.asarray(key_in[b]).T).astype(bf)
        xt[("v", b)] = np.ascontiguousarray(np.asarray(value[b]).T).astype(bf)
    in_maps = []
    for c in range(N_CORES):
        b, g = c // 4, c % 4
        cols = slice(M * g, M * (g + 1))
        in_maps.append(
            {
                "xtq": xt[("q", b)],
                "xtk": xt[("k", b)],
                "xtv": xt[("v", b)],
                "wq": np.ascontiguousarray(np.asarray(Wq, np.float32)[:, cols]).astype(bf),
                "wk": np.ascontiguousarray(np.asarray(Wk, np.float32)[:, cols]).astype(bf),
                "wv": np.ascontiguousarray(np.asarray(Wv, np.float32)[:, cols]).astype(bf),
                "wo": np.ascontiguousarray(np.asarray(Wo, np.float32)[cols, :]).astype(bf),
                "masks": masks_np,
                "rmsv": rmsv_np,
            }
        )
    return in_maps


def lambda_full(lambda_q1, lambda_q2, lambda_k1, lambda_k2) -> float:
    l1 = np.exp(np.sum(np.asarray(lambda_q1, np.float32) * np.asarray(lambda_k1, np.float32)))
    l2 = np.exp(np.sum(np.asarray(lambda_q2, np.float32) * np.asarray(lambda_k2, np.float32)))
    return float(l1 - l2 + LAMBDA_INIT)


def kernel(query, key_in, value, Wq, Wk, Wv, Wo,
           lambda_q1, lambda_q2, lambda_k1, lambda_k2, rms_weight):
    lam = lambda_full(lambda_q1, lambda_q2, lambda_k1, lambda_k2)
    in_maps = prep_inputs(query, key_in, value, Wq, Wk, Wv, Wo, rms_weight)
    nc = build_bass(lam)
    res = run_bass_kernel_spmd(nc, in_maps, list(range(N_CORES)))
    out = np.zeros((B, T, D), np.float32)
    for c in range(N_CORES):
        out[c // 4] += res.results[c]["out"]
    return out



# revision 8
# speedup vs baseline: 1.3753x; 1.3753x over previous
"""Bass/Tile Trainium2 kernel for DiffMultiHeadedAttention.

Sharding: data-parallel over batch (B=2), tensor-parallel over heads
(16 heads -> 4 heads per core), 8 cores total. Each core computes the
QKV projections for its 4 heads, two-branch causal attention with the
differential combine + per-head RMS norm, and its partial output
projection; the host sums the 4 per-core partials of each batch.

v2 structure (HAM-warm schedule):
- Scores are software-pipelined one k-tile ahead of AV/denominator
  matmuls so the PE never waits on the Scalar-engine exp.
- The per-(h,j) epilogue is delayed one iteration via emission hooks;
  its PE broadcasts execute with all dependencies long resolved.
- Softmax reciprocals and the RMS rsqrt run on the Scalar engine as
  exp(-ln(x)) / exp(-0.5*ln(x)) -- both functions live in the single
  `natural_log_exp_and_others` table set, so no table reloads, and the
  slow iterative DVE reciprocal is never used.
- lambda is folded into the branch-2 denominator matmul (bf16 ones/|l|
  weights) with an exact fp32 correction in the exp bias; the rmsnorm
  affine weight and (1-lambda_init) are folded into Wo on the host.
- All broadcast-matmul moving operands are bf16 (fp32 rhs streams at
  half rate).
"""

import math
import sys

import numpy as np

sys.path.insert(0, "/opt/trn_rl_repo")

import ml_dtypes  # noqa: E402

import concourse.bass as bass  # noqa: E402
import concourse.tile as tile  # noqa: E402
from concourse import mybir  # noqa: E402
from concourse.bass_utils import run_bass_kernel_spmd  # noqa: E402

B, T, D = 2, 2048, 2048
H = 16
HD = 128
HALF = 64
DEPTH = 12
LAMBDA_INIT = 0.8 - 0.6 * math.exp(-0.3 * DEPTH)
EPS = 1e-5
SCALE = 1.0 / math.sqrt(HALF)

HPC = 4          # heads per core
M = HPC * HD     # 512: per-core projection width
N_CORES = 8

BF16 = mybir.dt.bfloat16
F32 = mybir.dt.float32
AF = mybir.ActivationFunctionType


def _split_sync_waits(nc: bass.Bass, limit: int = 1) -> int:
    """The walrus build in this container rejects instructions carrying
    more than one sem wait (setupSyncWait: "Too many sync wait
    commands"). Move excess waits onto same-engine nops inserted just
    before the instruction — the engine stalls at the nops instead, so
    semantics are unchanged."""
    import bass_rust as _br

    ctr = 0
    for fn in nc.m.functions:
        for blk in fn.blocks:
            insts = blk.instructions
            out = []
            changed = False
            for ins in insts:
                si = ins.sync_info
                waits = list(si.on_wait) if si is not None else []
                if len(waits) > limit and str(ins.engine) != "EngineType.Unassigned":
                    changed = True
                    for w in waits[:-limit]:
                        ctr += 1
                        nop = _br.InstNoOp(name=f"waitsplit-{ctr}", ins=[], outs=[])
                        nop.engine = ins.engine
                        nop.sync_info = _br.SyncInfo(on_wait=[w], on_update=[])
                        nc.register_instruction(nop, overwrite=True)
                        out.append(nop)
                    ins.sync_info = _br.SyncInfo(
                        on_wait=waits[-limit:], on_update=list(si.on_update)
                    )
                out.append(ins)
            if changed:
                insts[:] = out
    return ctr


def build_bass(lam: float, t: int = T) -> bass.Bass:
    """Emit the per-core program. `t` is the sequence length (2048 in
    production; smaller for simulator checks). Requires t % 512 == 0."""
    assert t % 512 == 0
    njb = t // 512     # tq blocks of 512
    nkt = t // 128     # tk tiles of 128
    nkd = D // 128     # contraction tiles for the projections (16)

    # branch-2 lambda folding: dps2 weights are 1/|lam| in bf16; the
    # residual (from bf16 rounding and the sign) is corrected exactly in
    # the exp bias: exp(-ln(d2*c) + ln(|lam|*c)) = |lam|/d2.
    use2 = abs(lam) > 1e-12
    if use2:
        c_bf = float(np.float32(1.0 / abs(lam)).astype(ml_dtypes.bfloat16))
        bias2 = float(np.log(np.float64(abs(lam)) * np.float64(c_bf)))
    else:
        c_bf, bias2 = 1.0, 0.0
    sub2 = lam > 0  # comb = m1 - m2 if lam>0 else m1 + m2

    nc = bass.Bass()

    xtq = nc.dram_tensor("xtq", [D, t], BF16, kind="ExternalInput")
    xtk = nc.dram_tensor("xtk", [D, t], BF16, kind="ExternalInput")
    xtv = nc.dram_tensor("xtv", [D, t], BF16, kind="ExternalInput")
    wq = nc.dram_tensor("wq", [D, M], BF16, kind="ExternalInput")
    wk = nc.dram_tensor("wk", [D, M], BF16, kind="ExternalInput")
    wv = nc.dram_tensor("wv", [D, M], BF16, kind="ExternalInput")
    wo = nc.dram_tensor("wo", [M, D], BF16, kind="ExternalInput")
    masks = nc.dram_tensor("masks", [128, 128], BF16, kind="ExternalInput")
    out = nc.dram_tensor("out", [t, D], F32, kind="ExternalOutput")

    with tile.TileContext(nc) as tc:
        with (
            tc.tile_pool(name="persist", bufs=1) as persist,
            tc.tile_pool(name="consts", bufs=1) as consts,
        ):
            ones_b = consts.tile([128, 1], BF16)
            nc.gpsimd.memset(ones_b[:], 1.0)
            lamr_b = consts.tile([128, 1], BF16)
            nc.gpsimd.memset(lamr_b[:], c_bf)
            onesr_b = consts.tile([1, 128], BF16)
            nc.gpsimd.memset(onesr_b[:], 1.0)
            eps_t = consts.tile([1, 1], F32)
            nc.gpsimd.memset(eps_t[:], EPS)
            bias2_t = consts.tile([1, 1], F32)
            nc.gpsimd.memset(bias2_t[:], bias2)
            mask_t = consts.tile([128, 128], BF16)
            nc.sync.dma_start(mask_t[:], masks[:])

            # persistent activations
            qt = persist.tile([128, HPC, njb, 512], BF16)   # [hd, h, jb, tq]
            kt = persist.tile([128, HPC, njb, 512], BF16)   # [hd, h, jb, tk]
            v = persist.tile([128, nkt, M], BF16)           # [t, ktile, m]
            attn = persist.tile([128, HPC, njb, 512], BF16)  # [hd, h, jb, tq]

            # ---- Phase A: projections ----
            with (
                tc.tile_pool(name="wp", bufs=2) as wpool,
                tc.tile_pool(name="xp", bufs=3) as xpool,
                tc.tile_pool(name="pa", bufs=4, space=bass.MemorySpace.PSUM) as pa,
            ):
                first_proj = True
                for wdram, xdram, dest, mode in (
                    (wv, xtv, v, "N"),
                    (wk, xtk, kt, "T"),
                    (wq, xtq, qt, "T"),
                ):
                  with nc.named_scope(f"proj_{mode}_{wdram.name}"):
                      w_sb = wpool.tile([128, nkd, M], BF16, tag="w")
                      if first_proj:
                          # chunked so the first matmul can start early
                          for kc in range(4):
                              nc.sync.dma_start(
                                  w_sb[:, 4 * kc : 4 * (kc + 1), :],
                                  wdram[512 * kc : 512 * (kc + 1), :].rearrange(
                                      "(k p) m -> p k m", p=128
                                  ),
                              )
                      else:
                          nc.sync.dma_start(
                              w_sb[:], wdram.rearrange("(k p) m -> p k m", p=128)
                          )
                      for jb in range(njb):
                          x_sb = xpool.tile([128, nkd, 512], BF16, tag="xt")
                          xsrc = xdram[:, 512 * jb : 512 * (jb + 1)]
                          if first_proj and jb == 0:
                              for kc in range(4):
                                  nc.sync.dma_start(
                                      x_sb[:, 4 * kc : 4 * (kc + 1), :],
                                      xsrc[512 * kc : 512 * (kc + 1), :].rearrange(
                                          "(k p) t -> p k t", p=128
                                      ),
                                  )
                          else:
                              nc.sync.dma_start(
                                  x_sb[:], xsrc.rearrange("(k p) t -> p k t", p=128)
                              )
                          if mode == "T":
                              for mt in range(HPC):
                                  ps = pa.tile([128, 512], F32, tag="pa")
                                  for k in range(nkd):
                                      nc.tensor.matmul(
                                          ps[:],
                                          lhsT=w_sb[:, k, 128 * mt : 128 * (mt + 1)],
                                          rhs=x_sb[:, k, :],
                                          start=(k == 0),
                                          stop=(k == nkd - 1),
                                      )
                                  nc.vector.tensor_copy(dest[:, mt, jb, :], ps[:])
                          else:
                              for tw in range(4):
                                  ps = pa.tile([128, 512], F32, tag="pa")
                                  for k in range(nkd):
                                      nc.tensor.matmul(
                                          ps[:],
                                          lhsT=x_sb[:, k, 128 * tw : 128 * (tw + 1)],
                                          rhs=w_sb[:, k, :],
                                          start=(k == 0),
                                          stop=(k == nkd - 1),
                                      )
                                  nc.vector.tensor_copy(v[:, 4 * jb + tw, :], ps[:])
                          first_proj = False

            # ---- Phase B+C: attention, delayed epilogue, outproj ----
            with (
                tc.tile_pool(name="ep", bufs=1) as epool,
                tc.tile_pool(name="mt", bufs=4) as mpool,
                tc.tile_pool(name="ft", bufs=2) as ftmp,
                tc.tile_pool(name="sm", bufs=2) as smtmp,
                tc.tile_pool(name="wop", bufs=1) as wop,
                tc.tile_pool(name="ob", bufs=2) as ob,
                tc.tile_pool(name="pss", bufs=1, space=bass.MemorySpace.PSUM) as pss,
                tc.tile_pool(name="psatt", bufs=2, space=bass.MemorySpace.PSUM) as psatt,
                tc.tile_pool(name="psd", bufs=2, space=bass.MemorySpace.PSUM) as psd,
                tc.tile_pool(name="psw", bufs=2, space=bass.MemorySpace.PSUM) as psw,
            ):
                wo_sb = wop.tile([128, HPC, D], BF16)
                nc.sync.dma_start(wo_sb[:], wo.rearrange("(h p) n -> p h n", p=128))

                def make_epi(h, j, m1, m2, dps):
                    """Epilogue for iteration (h,j), emitted during the NEXT
                    iteration via hooks. Returns {point: fn}."""
                    st = {}

                    def act1():
                        sc = nc.enter_named_scope("epi_act", False)
                        r1l = smtmp.tile([1, 512], F32, tag="r1l")
                        nc.scalar.activation(r1l[:], dps[0:1, :], AF.Ln)
                        r1e = smtmp.tile([1, 512], BF16, tag="r1e")
                        nc.scalar.activation(r1e[:], r1l[:], AF.Exp, scale=-1.0)
                        st["r1e"] = r1e
                        nc.leave_named_scope("epi_act", sc[0], False)

                    def act2():
                        if not use2:
                            return
                        sc = nc.enter_named_scope("epi_act", False)
                        r2l = smtmp.tile([1, 512], F32, tag="r2l")
                        nc.scalar.activation(r2l[:], dps[32:33, :], AF.Ln)
                        r2e = smtmp.tile([1, 512], BF16, tag="r2e")
                        nc.scalar.activation(
                            r2e[:], r2l[:], AF.Exp, scale=-1.0, bias=bias2_t[:]
                        )
                        st["r2e"] = r2e
                        nc.leave_named_scope("epi_act", sc[0], False)

                    def mm1():
                        sc = nc.enter_named_scope("epi_mm", False)
                        rb1 = psw.tile([128, 512], F32, tag="w")
                        nc.tensor.matmul(
                            rb1[:], lhsT=onesr_b[:], rhs=st["r1e"][:],
                            start=True, stop=True,
                        )
                        nc.vector.tensor_mul(m1[:], m1[:], rb1[:])
                        nc.leave_named_scope("epi_mm", sc[0], False)

                    def mm2():
                        sc = nc.enter_named_scope("epi_mm", False)
                        comb = ftmp.tile([128, 512], F32, tag="comb")
                        if use2:
                            rb2 = psw.tile([128, 512], F32, tag="w")
                            nc.tensor.matmul(
                                rb2[:], lhsT=onesr_b[:], rhs=st["r2e"][:],
                                start=True, stop=True,
                            )
                            nc.vector.tensor_mul(m2[:], m2[:], rb2[:])
                            if sub2:
                                nc.vector.tensor_sub(comb[:], m1[:], m2[:])
                            else:
                                nc.vector.tensor_add(comb[:], m1[:], m2[:])
                        else:
                            nc.vector.tensor_copy(comb[:], m1[:])
                        sq = ftmp.tile([128, 512], BF16, tag="sq")
                        nc.vector.tensor_mul(sq[:], comb[:], comb[:])
                        st["comb"] = comb
                        st["sq"] = sq
                        nc.leave_named_scope("epi_mm", sc[0], False)

                    def mm3():
                        sc = nc.enter_named_scope("epi_mm", False)
                        nc.tensor.matmul(
                            dps[64:65, :], lhsT=ones_b[:], rhs=st["sq"][:],
                            start=True, stop=True, tile_position=(0, 64),
                        )
                        svl = smtmp.tile([1, 512], F32, tag="svl")
                        nc.scalar.activation(
                            svl[:], dps[64:65, :], AF.Ln,
                            scale=1.0 / 128.0, bias=eps_t[:],
                        )
                        fac = smtmp.tile([1, 512], BF16, tag="fac")
                        nc.scalar.activation(fac[:], svl[:], AF.Exp, scale=-0.5)
                        fb = psw.tile([128, 512], F32, tag="w")
                        nc.tensor.matmul(
                            fb[:], lhsT=onesr_b[:], rhs=fac[:],
                            start=True, stop=True,
                        )
                        nc.vector.tensor_mul(attn[:, h, j, :], st["comb"][:], fb[:])
                        nc.leave_named_scope("epi_mm", sc[0], False)

                    return {"exp0": act1, "exp1": act2, "i2_2": mm1,
                            "i2_3": mm2, "end": mm3}

                def emit_outproj(j):
                    sc_o = nc.enter_named_scope("outproj", False)
                    for tw in range(4):
                        tt = 4 * j + tw
                        o_sb = ob.tile([128, D], F32, tag="o")
                        for db in range(4):
                            po = psw.tile([128, 512], F32, tag="w")
                            for h in range(HPC):
                                nc.tensor.matmul(
                                    po[:],
                                    lhsT=attn[:, h, j, 128 * tw : 128 * (tw + 1)],
                                    rhs=wo_sb[:, h, 512 * db : 512 * (db + 1)],
                                    start=(h == 0),
                                    stop=(h == HPC - 1),
                                )
                            nc.vector.tensor_copy(
                                o_sb[:, 512 * db : 512 * (db + 1)], po[:]
                            )
                        nc.sync.dma_start(out[128 * tt : 128 * (tt + 1), :], o_sb[:])
                    nc.leave_named_scope("outproj", sc_o[0], False)

                def run_hooks(point):
                    fns = pending.get(point)
                    if fns is None:
                        return
                    if callable(fns):
                        fns()
                    else:
                        for fn in fns:
                            fn()

                pending = {}
                for j in range(njb):
                    for h in range(HPC):
                        ntk = 4 * j + 4
                        # [hd, tk-tile, branch, tq]
                        ea = epool.tile([128, nkt, 2, 512], BF16, tag="e")
                        att1 = psatt.tile([128, 512], F32, tag="att")
                        att2 = psatt.tile([128, 512], F32, tag="att")
                        dps = psd.tile([128, 512], F32, tag="d")

                        def emit_sc(i2):
                            sc_s = nc.enter_named_scope("att_s", False)
                            sps = pss.tile([128, 2, 512], F32, tag="s")
                            nc.tensor.matmul(
                                sps[:, 0, :],
                                lhsT=kt[0:64, h, i2 // 4, 128 * (i2 % 4) : 128 * (i2 % 4 + 1)],
                                rhs=qt[0:64, h, j, :],
                                start=True,
                                stop=True,
                            )
                            nc.tensor.matmul(
                                sps[:, 1, :],
                                lhsT=kt[64:128, h, i2 // 4, 128 * (i2 % 4) : 128 * (i2 % 4 + 1)],
                                rhs=qt[64:128, h, j, :],
                                start=True,
                                stop=True,
                            )
                            nc.leave_named_scope("att_s", sc_s[0], False)
                            return sps

                        sps_cur = emit_sc(0)
                        for i2 in range(ntk):
                            r = i2 - 4 * j
                            off = 128 * r if r > 0 else 0
                            sc_e = nc.enter_named_scope("att_exp", False)
                            nc.scalar.activation(
                                ea[:, i2, :, off:512], sps_cur[:, :, off:512],
                                AF.Exp, scale=SCALE,
                            )
                            nc.leave_named_scope("att_exp", sc_e[0], False)
                            if i2 == 0:
                                run_hooks("exp0")
                            elif i2 == 1:
                                run_hooks("exp1")
                            if i2 + 1 < ntk:
                                sps_cur = emit_sc(i2 + 1)
                            if i2 == 2:
                                run_hooks("i2_2")
                            elif i2 == 3:
                                run_hooks("i2_3")
                            if r >= 0:
                                sc_m = nc.enter_named_scope("att_mask", False)
                                nc.vector.tensor_mul(
                                    ea[:, i2, 0, off : off + 128],
                                    ea[:, i2, 0, off : off + 128],
                                    mask_t[:],
                                )
                                nc.vector.tensor_mul(
                                    ea[:, i2, 1, off : off + 128],
                                    ea[:, i2, 1, off : off + 128],
                                    mask_t[:],
                                )
                                nc.leave_named_scope("att_mask", sc_m[0], False)
                            sc_a = nc.enter_named_scope("att_av", False)
                            nc.tensor.matmul(
                                att1[:, off:512],
                                lhsT=v[:, i2, 128 * h : 128 * (h + 1)],
                                rhs=ea[:, i2, 0, off:512],
                                start=(i2 == 0),
                                stop=(i2 == ntk - 1),
                            )
                            nc.tensor.matmul(
                                att2[:, off:512],
                                lhsT=v[:, i2, 128 * h : 128 * (h + 1)],
                                rhs=ea[:, i2, 1, off:512],
                                start=(i2 == 0),
                                stop=(i2 == ntk - 1),
                            )
                            nc.leave_named_scope("att_av", sc_a[0], False)
                            sc_d = nc.enter_named_scope("att_d", False)
                            nc.tensor.matmul(
                                dps[0:1, off:512],
                                lhsT=ones_b[:],
                                rhs=ea[:, i2, 0, off:512],
                                start=(i2 == 0),
                                stop=(i2 == ntk - 1),
                                tile_position=(0, 0),
                            )
                            nc.tensor.matmul(
                                dps[32:33, off:512],
                                lhsT=lamr_b[:],
                                rhs=ea[:, i2, 1, off:512],
                                start=(i2 == 0),
                                stop=(i2 == ntk - 1),
                                tile_position=(0, 32),
                            )
                            nc.leave_named_scope("att_d", sc_d[0], False)

                        run_hooks("end")

                        # evacuate att psums (ACT for one, DVE for the other)
                        sc_v = nc.enter_named_scope("evac", False)
                        m1 = mpool.tile([128, 512], F32, tag="m")
                        nc.scalar.copy(m1[:], att1[:])
                        m2 = mpool.tile([128, 512], F32, tag="m")
                        nc.vector.tensor_copy(m2[:], att2[:])
                        nc.leave_named_scope("evac", sc_v[0], False)

                        pending = make_epi(h, j, m1, m2, dps)
                        if h == HPC - 1:
                            # outproj(j) must follow the delayed attn write
                            pending["end"] = (
                                pending["end"],
                                lambda jj=j: emit_outproj(jj),
                            )

                # tail: final iteration's epilogue + outproj
                for point in ("exp0", "exp1", "i2_2", "i2_3", "end"):
                    run_hooks(point)

    _split_sync_waits(nc)
    return nc


def make_masks() -> np.ndarray:
    tk = np.arange(128)[:, None]
    tq = np.arange(128)[None, :]
    return (tq >= tk).astype(ml_dtypes.bfloat16)


def prep_inputs(query, key_in, value, Wq, Wk, Wv, Wo, rms_weight, t=T):
    """Build the 8 per-core input maps (host-side sharding + layout prep).
    rms_weight * (1 - LAMBDA_INIT) is folded into Wo's rows."""
    bf = ml_dtypes.bfloat16
    masks_np = make_masks()
    rms_fold = (
        np.tile(np.asarray(rms_weight, np.float32), HPC) * (1.0 - LAMBDA_INIT)
    )[:, None]  # [512, 1] per-core Wo row scale
    xt = {}
    for b in range(B):
        xt[("q", b)] = np.ascontiguousarray(np.asarray(query[b]).T).astype(bf)
        xt[("k", b)] = np.ascontiguousarray(np.asarray(key_in[b]).T).astype(bf)
        xt[("v", b)] = np.ascontiguousarray(np.asarray(value[b]).T).astype(bf)
    in_maps = []
    for c in range(N_CORES):
        b, g = c // 4, c % 4
        cols = slice(M * g, M * (g + 1))
        wo_c = np.asarray(Wo, np.float32)[cols, :] * rms_fold
        in_maps.append(
            {
                "xtq": xt[("q", b)],
                "xtk": xt[("k", b)],
                "xtv": xt[("v", b)],
                "wq": np.ascontiguousarray(np.asarray(Wq, np.float32)[:, cols]).astype(bf),
                "wk": np.ascontiguousarray(np.asarray(Wk, np.float32)[:, cols]).astype(bf),
                "wv": np.ascontiguousarray(np.asarray(Wv, np.float32)[:, cols]).astype(bf),
                "wo": np.ascontiguousarray(wo_c).astype(bf),
                "masks": masks_np,
            }
        )
    return in_maps


def lambda_full(lambda_q1, lambda_q2, lambda_k1, lambda_k2) -> float:
    l1 = np.exp(np.sum(np.asarray(lambda_q1, np.float32) * np.asarray(lambda_k1, np.float32)))
    l2 = np.exp(np.sum(np.asarray(lambda_q2, np.float32) * np.asarray(lambda_k2, np.float32)))
    return float(l1 - l2 + LAMBDA_INIT)


def kernel(query, key_in, value, Wq, Wk, Wv, Wo,
           lambda_q1, lambda_q2, lambda_k1, lambda_k2, rms_weight):
    lam = lambda_full(lambda_q1, lambda_q2, lambda_k1, lambda_k2)
    in_maps = prep_inputs(query, key_in, value, Wq, Wk, Wv, Wo, rms_weight)
    nc = build_bass(lam)
    res = run_bass_kernel_spmd(nc, in_maps, list(range(N_CORES)))
    out = np.zeros((B, T, D), np.float32)
    for c in range(N_CORES):
        out[c // 4] += res.results[c]["out"]
    return out
